# revision 26
# baseline (speedup 1.0000x reference)
"""ALiBi multi-head attention on 8 TRN2 NeuronCores.

Sharding: data-parallel over batch (B=2 -> 2 groups of 4 cores), tensor-parallel
over the 16 heads (4 heads per core, Megatron-style column shards of Wq/Wk/Wv).
The output projection uses per-head-slot AllGathers of the normalized attention
outputs inside each 4-core group, overlapped with the remaining slots' attention
compute, followed by a column shard of Wo on every core accumulated slot by
slot into SBUF (cheaper than the row-shard + AllReduce formulation and hides
most of the collective latency).

Head assignment is slot-ranked: core group-position j gets heads
{15-j, 11-j, 7-j, 3-j}. All cores run one SPMD instruction stream, so the
ALiBi band schedule of slot s is sized for the widest head in that slot; the
per-core ALiBi factor tiles F = exp(-slope*|k-q|) (fp16, Toeplitz: one tile per
tile-diagonal offset) carry each core's actual slopes and double as the band
mask (F underflows to exactly 0 outside the head's own band).

On-chip layout: activations are passed transposed ([D, S]) so the Q/K
projections directly produce Q^T/K^T ([head_dim, S]) -- the layout needed by
scoresT = (K^T).T @ Q^T -- while V is produced in natural [S, head_dim] layout
with a fused ones-column so the attention-value matmul also emits the softmax
denominators (row 64) for free. Softmax skips the max-subtraction: for this
problem's scale (scores ~ N(0, 0.45^2), max < 3) fp32 exp cannot overflow.
"""

import math
import sys

import numpy as np

if "/opt/trn_rl_repo" not in sys.path:
    sys.path.insert(0, "/opt/trn_rl_repo")

import concourse.bass as bass  # noqa: E402
import concourse.mybir as mybir  # noqa: E402
import concourse.tile as tile  # noqa: E402
from concourse import bacc  # noqa: E402
from concourse.bass_utils import run_bass_kernel_spmd  # noqa: E402

B, S, D, H, HD = 2, 2048, 1024, 16, 64
NCORES = 8
GROUP = 4          # cores per batch group
NSLOT = 4          # head slots per core
KT = 128           # k (key position) tile size
NKT = S // KT      # 16
NDC = D // 128     # 8 contraction chunks
T_FACTOR = 15.0    # band radius = T_FACTOR / slope  (exp(-15) ~ 3e-7)

F16 = mybir.dt.float16
F32 = mybir.dt.float32

SLOPES = [2.0 ** (-0.5 * (h + 1)) for h in range(H)]
# BINS[j][s] = head of slot s on cores j and j+4
BINS = [[15 - j, 11 - j, 7 - j, 3 - j] for j in range(GROUP)]
# slots processed narrow-band first so attention starts before all
# projections are done and wide slots overlap the per-slot collectives
SLOT_ORDER = [3, 2, 1, 0]


def _slot_nd():
    """Max |k_tile - q_tile| included per slot (widest head in the slot)."""
    nds = []
    for s in range(NSLOT):
        t = max(
            min(S - 1, int(math.ceil(T_FACTOR / SLOPES[BINS[j][s]])))
            for j in range(GROUP)
        )
        nds.append(min(NKT - 1, (t + KT - 1) // KT))
    return nds


SLOT_ND = _slot_nd()
# q tiles are processed in aligned groups of 4 (one [65,512] AV psum tile per
# group); padding a group can reference offsets up to nd+3 away, so F tiles
# extend that far (values there underflow to 0 in fp16 -> free masking).
SLOT_NDE = [min(NKT - 1, nd + (7 if nd >= 8 else 3)) for nd in SLOT_ND]
F_BASE = []
_acc = 0
for _s in range(NSLOT):
    F_BASE.append(_acc)
    _acc += 2 * SLOT_NDE[_s] + 1
NF = _acc  # total ALiBi factor tiles per core


def _f_idx(s, dd):
    """Index of the F tile for slot s, diagonal offset dd = k_tile - q_tile.

    Stored so that for fixed k_tile, consecutive q_tiles read consecutive
    F tiles (lets one tensor_mul cover a whole q-chunk).
    """
    return F_BASE[s] + SLOT_NDE[s] - dd


def build_graph():
    nc = bacc.Bacc("TRN2", target_bir_lowering=False, debug=False,
                   num_devices=NCORES)
    xq = nc.dram_tensor("xq", [128, NDC * S], F16, kind="ExternalInput")
    xk = nc.dram_tensor("xk", [128, NDC * S], F16, kind="ExternalInput")
    xv = nc.dram_tensor("xv", [128, NDC * S], F16, kind="ExternalInput")
    wq = nc.dram_tensor("wq", [128, NDC * 256], F16, kind="ExternalInput")
    wk = nc.dram_tensor("wk", [128, NDC * 256], F16, kind="ExternalInput")
    wv = nc.dram_tensor("wv", [128, NDC * 256], F16, kind="ExternalInput")
    wo = nc.dram_tensor("wo", [128, NDC * 256], F16, kind="ExternalInput")
    fm = nc.dram_tensor("fm", [128, NF * KT], F16, kind="ExternalInput")
    out = nc.dram_tensor("out", [S, 256], F32, kind="ExternalOutput")

    with tile.TileContext(nc) as tc:
        with (
            tc.tile_pool(name="wpool", bufs=1) as wpool,
            tc.tile_pool(name="persist", bufs=1) as persist,
            tc.tile_pool(name="xpool", bufs=1) as xpool,
            tc.tile_pool(name="dram", bufs=1, space="DRAM") as dram,
            tc.tile_pool(name="prps", bufs=2, space="PSUM") as prps,
            tc.tile_pool(name="scps", bufs=1, space="PSUM") as scps,
            tc.tile_pool(name="avps", bufs=1, space="PSUM") as avps,
            tc.tile_pool(name="psb", bufs=3) as psb,
            tc.tile_pool(name="nsb", bufs=2) as nsb,
        ):
            wv_sb = wpool.tile([128, NDC * 256], F16)
            wq_sb = wpool.tile([128, NDC * 256], F16)
            wk_sb = wpool.tile([128, NDC * 256], F16)
            f_sb = wpool.tile([128, NF * KT], F16)
            wo_sb = wpool.tile([128, NDC * 256], F16)

            def load_x(src, nm):
                tiles = []
                for c in range(NDC):
                    t = xpool.tile([128, S], F16, tag="x", bufs=2 * NDC,
                                   name=f"{nm}{c}")
                    nc.sync.dma_start(t[:], src[:, c * S:(c + 1) * S])
                    tiles.append(t)
                return tiles

            # Q^T/K^T: two slots per 128-partition tile (slot s%2 at
            # partition 64*(s%2)); V natural layout, per k-tile, with a ones
            # column per (k_tile, slot) at vx[kt][:, s*65 + 64].
            qt_sb = [persist.tile([128, S], F16, name=f"qt{m}") for m in range(2)]
            kt_sb = [persist.tile([128, S], F16, name=f"kt{m}") for m in range(2)]
            vx_sb = [persist.tile([128, NSLOT * 65], F16, name=f"vx{k}")
                     for k in range(NKT)]
            ones_sb = persist.tile([65, 64], F16)
            nc.vector.memset(ones_sb[:], 1.0)
            for k in range(NKT):
                nc.vector.memset(
                    vx_sb[k][:].rearrange("p (s e) -> p s e", s=NSLOT)[:, :, 64:65],
                    1.0,
                )
            normt_sb = [persist.tile([64, S], F16, name=f"nt{s}")
                        for s in range(NSLOT)]
            # Wo output accumulator (over slots), in SBUF
            oacc_sb = persist.tile([128, 16 * 256], F32)

            # ---- projections -------------------------------------------------
            # V first (attention's slot loop consumes vx per k tile)
            nc.sync.dma_start(wv_sb[:], wv[:])
            xv_sb = load_x(xv, "xv")
            nc.sync.dma_start(wk_sb[:], wk[:])
            nc.sync.dma_start(wq_sb[:], wq[:])
            for kt in range(NKT):
                ps = prps.tile([128, 256], F32, tag="pr")
                for c in range(NDC):
                    nc.tensor.matmul(
                        ps[:],
                        lhsT=xv_sb[c][:, kt * 128:kt * 128 + 128],
                        rhs=wv_sb[:, c * 256:(c + 1) * 256],
                        start=(c == 0),
                        stop=(c == NDC - 1),
                    )
                for s in range(NSLOT):
                    nc.vector.tensor_copy(
                        vx_sb[kt][:, s * 65:s * 65 + 64],
                        ps[:, s * 64:(s + 1) * 64],
                    )

            # K^T / Q^T projections, one (w, x, mb, nb) block at a time.
            # mb=1 (slots 2,3) is emitted up front; the mb=0 blocks are
            # deferred into the early attention slots to keep the PE dense
            # (and hot) while attention is ACT-bound.
            def qk_block(wsb, xsb, dsts, mb, nb):
                ps = prps.tile([128, 512], F32, tag="pr")
                for c in range(NDC):
                    nc.tensor.matmul(
                        ps[:],
                        lhsT=wsb[:, c * 256 + mb * 128:c * 256 + mb * 128 + 128],
                        rhs=xsb[c][:, nb * 512:(nb + 1) * 512],
                        start=(c == 0),
                        stop=(c == NDC - 1),
                    )
                nc.vector.tensor_copy(
                    dsts[mb][:, nb * 512:(nb + 1) * 512], ps[:]
                )

            xk_sb = load_x(xk, "xk")
            nc.sync.dma_start(f_sb[:], fm[:])
            xq_sb = load_x(xq, "xq")
            nc.sync.dma_start(wo_sb[:], wo[:])
            for nb in range(4):
                qk_block(wk_sb, xk_sb, kt_sb, 1, nb)
            for nb in range(4):
                qk_block(wq_sb, xq_sb, qt_sb, 1, nb)
            side_work = [lambda nb=nb: qk_block(wk_sb, xk_sb, kt_sb, 0, nb)
                         for nb in range(4)]
            side_work += [lambda nb=nb: qk_block(wq_sb, xq_sb, qt_sb, 0, nb)
                          for nb in range(4)]

            # ---- attention + collectives + Wo accumulation -------------------
            # Each slot's AllGather is split into two q-halves so the last
            # half's collective + Wo chunk is the only exposed tail.
            HS = S // 2
            agin = {}
            agout = {}
            for s_ in range(NSLOT):
                for h_ in range(2):
                    agin[s_, h_] = dram.tile([64, HS], F16,
                                             name=f"agin{s_}_{h_}")
                    agout[s_, h_] = dram.tile([GROUP * 64, HS], F16,
                                              name=f"agout{s_}_{h_}")

            def emit_ag(s, h):
                nc.sync.dma_start(agin[s, h][:],
                                  normt_sb[s][:, h * HS:(h + 1) * HS])
                nc.gpsimd.collective_compute(
                    "AllGather",
                    mybir.AluOpType.bypass,
                    ins=[agin[s, h].opt()],
                    outs=[agout[s, h].opt()],
                    replica_groups=[[0, 1, 2, 3], [4, 5, 6, 7]],
                )

            def emit_wo(s, h, first):
                # Wo contribution of half (s, h), accumulated into oacc_sb.
                # Emitted one half late so the in-order PE stream never waits
                # on the collective.
                gath = psb.tile([128, 2 * HS], F16, tag="gath", bufs=2,
                                name=f"gath{s}_{h}")
                nc.sync.dma_start(
                    gath[:].rearrange("p (c t) -> p c t", c=2),
                    agout[s, h][:].rearrange("(c p) t -> p c t", p=128),
                )
                for mi in range(8):
                    m = 8 * h + mi
                    po_ = prps.tile([128, 256], F32, tag="pr")
                    for c2 in range(2):
                        nc.tensor.matmul(
                            po_[:],
                            lhsT=gath[:, c2 * HS + mi * 128:c2 * HS + mi * 128 + 128],
                            rhs=wo_sb[:, (2 * s + c2) * 256:(2 * s + c2 + 1) * 256],
                            start=(c2 == 0),
                            stop=(c2 == 1),
                        )
                    if first:
                        nc.vector.tensor_copy(
                            oacc_sb[:, m * 256:(m + 1) * 256], po_[:]
                        )
                    else:
                        nc.vector.tensor_tensor(
                            oacc_sb[:, m * 256:(m + 1) * 256],
                            po_[:],
                            oacc_sb[:, m * 256:(m + 1) * 256],
                            mybir.AluOpType.add,
                        )

            wo_queue = []
            seen_half = [False, False]

            def flush_wo():
                s_, h_ = wo_queue.pop(0)
                emit_wo(s_, h_, first=not seen_half[h_])
                seen_half[h_] = True

            for si, s in enumerate(SLOT_ORDER):
                nd = SLOT_ND[s]
                po = 64 * (s % 2)
                qt_t = qt_sb[s // 2]
                kt_t = kt_sb[s // 2]
                units = [(0, 2), (2, 2)] if nd >= 8 else [(g, 1) for g in range(4)]
                for (g0, ng) in units:
                    k_lo = max(0, 4 * g0 - nd)
                    k_hi = min(NKT - 1, 4 * g0 + 4 * ng - 1 + nd)
                    avs = [avps.tile([65, 512], F32, tag="av", bufs=2,
                                     name=f"av{s}_{g0 + gi}")
                           for gi in range(ng)]
                    for kt in range(k_lo, k_hi + 1):
                        sc = scps.tile([128, ng * 512], F32, tag="sc", bufs=2)
                        for gi in range(ng):
                            nc.tensor.matmul(
                                sc[:, gi * 512:(gi + 1) * 512],
                                lhsT=kt_t[po:po + 64, kt * 128:kt * 128 + 128],
                                rhs=qt_t[po:po + 64,
                                         (g0 + gi) * 512:(g0 + gi + 1) * 512],
                            )
                        pt = psb.tile([128, ng * 512], F16, tag="pt")
                        nc.scalar.activation(
                            pt[:], sc[:], mybir.ActivationFunctionType.Exp
                        )
                        fi = _f_idx(s, kt - 4 * g0)
                        pm = psb.tile([128, ng * 512], F16, tag="pm")
                        nc.vector.tensor_mul(
                            pm[:], pt[:], f_sb[:, fi * 128:(fi + 4 * ng) * 128]
                        )
                        for gi in range(ng):
                            nc.tensor.matmul(
                                avs[gi][:],
                                lhsT=vx_sb[kt][:, s * 65:s * 65 + 65],
                                rhs=pm[:, gi * 512:(gi + 1) * 512],
                                start=(kt == k_lo),
                                stop=(kt == k_hi),
                            )
                    for gi in range(ng):
                        g = g0 + gi
                        # normalize group g
                        av_sb = nsb.tile([65, 512], F32, tag="avs")
                        nc.vector.tensor_copy(av_sb[:], avs[gi][:])
                        # denominator row -> [32,16] block at partition 0 so
                        # the reciprocal runs 32 lanes wide instead of 1
                        dsh = nsb.tile([32, 16], F32, tag="dsh")
                        nc.sync.dma_start(dsh[:], av_sb[64:65, :])
                        rec = nsb.tile([32, 16], F32, tag="rec")
                        nc.vector.reciprocal(rec[:], dsh[:])
                        rec16 = nsb.tile([32, 16], F16, tag="rec16")
                        nc.vector.tensor_copy(rec16[:], rec[:])
                        rrow = nsb.tile([1, 512], F16, tag="rrow")
                        nc.sync.dma_start(rrow[:], rec16[:])
                        bc = prps.tile([64, 512], F32, tag="pr",
                                       name=f"bc{s}_{g}")
                        nc.tensor.matmul(bc[:], lhsT=ones_sb[0:1, :],
                                         rhs=rrow[:])
                        nc.vector.tensor_mul(
                            normt_sb[s][:, g * 512:(g + 1) * 512],
                            av_sb[0:64, :],
                            bc[:],
                        )
                    # after the unit covering q-half h completes, fire that
                    # half's collective and emit the Wo chunk of the
                    # previously queued half
                    done_q = (g0 + ng) * 4  # q tiles completed
                    if done_q == 8 or done_q == 16:
                        h = 0 if done_q == 8 else 1
                        emit_ag(s, h)
                        if wo_queue:
                            flush_wo()
                        wo_queue.append((s, h))
                    if side_work:
                        side_work.pop(0)()

            while wo_queue:
                flush_wo()

            nc.sync.dma_start(
                out[:].rearrange("(m p) n -> p m n", p=128),
                oacc_sb[:].rearrange("p (m n) -> p m n", m=16),
            )

    nc.compile()
    return nc


_NC_CACHE = None


def _get_graph():
    global _NC_CACHE
    if _NC_CACHE is None:
        _NC_CACHE = build_graph()
    return _NC_CACHE


def _swizzle_cd(a):
    """[C*128, X] -> [128, C*X] with row p holding chunks c at [c*X:(c+1)*X]."""
    c = a.shape[0] // 128
    return np.ascontiguousarray(
        a.reshape(c, 128, a.shape[1]).transpose(1, 0, 2).reshape(128, -1)
    )


def _host_inputs(query, key, value, Wq, Wk, Wv, Wo):
    xqs, xks, xvs = [], [], []
    for b in range(B):
        xqs.append(_swizzle_cd(query[b].T.astype(np.float32)).astype(np.float16))
        xks.append(_swizzle_cd(key[b].T.astype(np.float32)).astype(np.float16))
        xvs.append(_swizzle_cd(value[b].T.astype(np.float32)).astype(np.float16))

    scale = 1.0 / math.sqrt(HD)
    wqs, wks, wvs, fms = [], [], [], []
    for j in range(GROUP):
        cols = np.concatenate(
            [np.arange(64 * h, 64 * h + 64) for h in BINS[j]]
        )
        wqs.append(_swizzle_cd((Wq[:, cols] * scale).astype(np.float32)).astype(np.float16))
        wks.append(_swizzle_cd(Wk[:, cols].astype(np.float32)).astype(np.float16))
        wvs.append(_swizzle_cd(Wv[:, cols].astype(np.float32)).astype(np.float16))

        f = np.zeros((128, NF * KT), np.float32)
        p = np.arange(128)[:, None]
        q = np.arange(128)[None, :]
        for s in range(NSLOT):
            sl = SLOPES[BINS[j][s]]
            for dd in range(-SLOT_NDE[s], SLOT_NDE[s] + 1):
                fi = _f_idx(s, dd)
                f[:, fi * 128:(fi + 1) * 128] = np.exp(
                    -sl * np.abs(dd * 128 + p - q)
                )
        fms.append(f.astype(np.float16))

    # Wo rows permuted to gathered order (slot-major):
    # row 256*s + 64*r + d  <->  original row 64*BINS[r][s] + d
    perm = np.concatenate(
        [np.arange(64 * BINS[r][s], 64 * BINS[r][s] + 64)
         for s in range(NSLOT) for r in range(GROUP)]
    )
    wos = []
    for j in range(GROUP):
        wos.append(
            _swizzle_cd(
                Wo[perm][:, 256 * j:256 * (j + 1)].astype(np.float32)
            ).astype(np.float16)
        )

    in_maps = []
    for i in range(NCORES):
        b, j = i // GROUP, i % GROUP
        in_maps.append({
            "xq": xqs[b], "xk": xks[b], "xv": xvs[b],
            "wq": wqs[j], "wk": wks[j], "wv": wvs[j], "wo": wos[j],
            "fm": fms[j],
        })
    return in_maps


def kernel(**inputs):
    query = np.asarray(inputs["query"], np.float32)
    key = np.asarray(inputs["key"], np.float32)
    value = np.asarray(inputs["value"], np.float32)
    Wq = np.asarray(inputs["Wq"], np.float32)
    Wk = np.asarray(inputs["Wk"], np.float32)
    Wv = np.asarray(inputs["Wv"], np.float32)
    Wo = np.asarray(inputs["Wo"], np.float32)

    nc = _get_graph()
    in_maps = _host_inputs(query, key, value, Wq, Wk, Wv, Wo)
    res = run_bass_kernel_spmd(nc, in_maps, list(range(NCORES)))

    full = np.empty((B, S, D), np.float32)
    for b in range(B):
        for j in range(GROUP):
            full[b][:, 256 * j:256 * (j + 1)] = res.results[GROUP * b + j]["out"]
    return full
